# revision 1
# baseline (speedup 1.0000x reference)
"""Trainium2 Bass kernel for nn_CountingAbstraction (sparse_attention).

Math (per batch b):
    cn  = l2_normalize(data[b], axis=-1)
    sim = relu(cn @ cn.T)                       # [N, N]
    counter_pre = sim @ [1 | fixed_v]           # rowsum + sim@posenc, [N, 513]
    counter = softplus(counter_pre @ W_exp + b_exp)
    out = [data | counter] @ W_merge

Device formulation (flash-attention-style fusion, never materializing sim):
    Wt = fixed_v @ W_exp[1:] + 1*W_exp[0]       # [N, M], folds rowsum+Dense
    z.T[m, q] = sum_k Wt[k, m] * relu(cnT_k.T @ cnT_q)[k, q]
    counter.T = softplus(z.T + b_exp)           # per-partition bias
    out[q, :] = dataT_q.T @ W_merge[:D] + counter.T.T @ W_merge[D:]

Sharding: core c handles batch c//2, query rows half c%2 (2048 rows) against
all 4096 keys of that batch. Data-parallel, no collectives.

Matmuls run in bf16 (fp32 PSUM accumulation). data arrives host-cast to bf16
(halves the startup DMA; norms computed from bf16 are within ~1e-4 because the
sum-of-squares averages 512 positive rounding errors). softplus is computed as
relu(z+b) [DVE] + ln(1 + exp(-|z+b|)) [ACT], which is range-safe, and the
merge matmuls of chunk ch-1 are emitted between the k-loop and softplus of
chunk ch so the in-order PE stream has work while ACT runs the softplus chain.
"""

import sys

for _p in ("/opt/trn_rl_repo",):
    if _p not in sys.path:
        sys.path.insert(0, _p)

import numpy as np
import ml_dtypes

import concourse.tile as tile
import concourse.mybir as mybir
from concourse import bacc
from concourse.bass import ts, ds
from concourse.bass_utils import run_bass_kernel_spmd

F32 = mybir.dt.float32
BF16 = mybir.dt.bfloat16
AF = mybir.ActivationFunctionType
ALU = mybir.AluOpType
BF = ml_dtypes.bfloat16

B, N, D, M = 4, 4096, 512, 512
NCORES = 8
NQ = (B * N) // NCORES  # 2048 query rows per core


def _posenc(n, d):
    pos = np.arange(n, dtype=np.float32)[:, None]
    i = np.arange(d // 2, dtype=np.float32)[None, :]
    angle = pos / np.power(10000.0, 2.0 * i / d)
    pe = np.zeros((n, d), dtype=np.float32)
    pe[:, 0::2] = np.sin(angle)
    pe[:, 1::2] = np.cos(angle)
    return pe


def _chunks(total, size):
    off = 0
    while off < total:
        w = min(size, total - off)
        yield off, w
        off += w


def build_nc(nkeys=N, nq=NQ, qch=512, num_cores=NCORES):
    """Build the SPMD Bass kernel (identical on every core)."""
    assert D % 128 == 0 and M % 128 == 0 and nkeys % 512 == 0
    assert nq % qch == 0 and qch % 128 == 0 and qch <= 512
    assert nq % 512 == 0  # rawq capture is per 512-column key chunk
    DP = D // 128       # contraction subtiles over feature dim
    MJ = M // 128       # output-column subtiles
    KB = nkeys // 128   # key blocks
    NCH = nq // qch     # query chunks

    nc = bacc.Bacc("TRN2", target_bir_lowering=False, debug=False,
                   num_devices=num_cores)
    dTk = nc.dram_tensor("dTk", [D, nkeys], BF16, kind="ExternalInput").ap()
    fvT = nc.dram_tensor("fvT", [D, nkeys], BF16, kind="ExternalInput").ap()
    wexp1 = nc.dram_tensor("wexp1", [D, M], BF16, kind="ExternalInput").ap()
    w0 = nc.dram_tensor("w0", [1, M], F32, kind="ExternalInput").ap()
    wm = nc.dram_tensor("wm", [D + M, M], BF16, kind="ExternalInput").ap()
    bexp = nc.dram_tensor("bexp", [MJ, 128], F32, kind="ExternalInput").ap()
    out = nc.dram_tensor("out", [nq, M], F32, kind="ExternalOutput").ap()

    with tile.TileContext(nc) as tc:
        with (
            tc.tile_pool(name="res", bufs=1) as res,
            tc.tile_pool(name="trans", bufs=2) as trans,
            tc.tile_pool(name="work", bufs=3) as work,
            tc.tile_pool(name="psg", bufs=3, space="PSUM") as psg,
            tc.tile_pool(name="psz", bufs=MJ, space="PSUM") as psz,
            tc.tile_pool(name="pso", bufs=1, space="PSUM") as pso,
        ):
            # ---- constants / residents -------------------------------------
            wexp_sb = res.tile([128, DP, M], BF16, tag="wexp", name="wexp_sb")
            for c in range(D // 128):
                nc.sync.dma_start(wexp_sb[:, c, :], wexp1[ts(c, 128), :])
            wm_sb = res.tile([128, DP + MJ, M], BF16, tag="wm", name="wm_sb")
            bexp_sb = res.tile([128, MJ], F32, tag="bexp", name="bexp_sb")
            nc.sync.dma_start(bexp_sb[:], bexp.rearrange("c p -> p c"))
            w0_sb = res.tile([1, M], F32, tag="w0", name="w0_sb")
            nc.sync.dma_start(w0_sb[:], w0[:])
            w0b = res.tile([128, M], F32, tag="w0b", name="w0b")
            nc.gpsimd.partition_broadcast(w0b[:], w0_sb[:])
            ones_col = res.tile([128, 1], BF16, tag="ones", name="ones_col")
            nc.vector.memset(ones_col[:], 1.0)

            wt = res.tile([128, KB, M], BF16, tag="wt", name="wt")
            cnk = res.tile([128, DP, nkeys], BF16, tag="cnk", name="cnk")
            rawq = res.tile([128, DP, nq], BF16, tag="rawq", name="rawq")
            n_row_k = res.tile([1, nkeys], F32, tag="nrk", name="n_row_k")

            # ---- phase A: Wt = fvT.T @ wexp1 + 1*w0 ------------------------
            # The first two groups are emitted up front (small DMAs unblock
            # dense PE work immediately); the rest are interleaved into the
            # norm passes below so norms matmuls fill the build's single-bank
            # WAR stalls and build matmuls fill the norm DMA waits.
            def emit_build_group(kg):
                fv2 = work.tile([128, DP, 512], BF16, tag="fv", bufs=2,
                                name="fv2")
                for c in range(DP):
                    nc.sync.dma_start(fv2[:, c, :], fvT[ts(c, 128), ts(kg, 512)])
                for k4 in range(4):
                    ki = kg * 4 + k4
                    pw = pso.tile([128, M], F32, tag="po", name="pw")
                    for c in range(DP):
                        nc.tensor.matmul(pw[:], fv2[:, c, ts(k4, 128)],
                                         wexp_sb[:, c, :],
                                         start=(c == 0), stop=(c == DP - 1))
                    nc.vector.tensor_add(wt[:, ki, :], pw[:], w0b[:])

            build_groups = iter(range(KB // 4))
            for _ in range(min(3, KB // 4)):
                emit_build_group(next(build_groups))

            def step_build(_ci):
                kg = next(build_groups, None)
                if kg is not None:
                    emit_build_group(kg)

            nc.sync.dma_start(wm_sb[:], wm.rearrange("(c p) m -> p c m", p=128))

            # ---- phase B: fused norms + normalized copies ------------------
            # One pass over the (bf16) data per 512-column chunk: sumsq via
            # square + matmul-with-ones, rsqrt, partition-broadcast, scale.
            # Query data lands directly in the resident rawq (it IS the raw
            # bf16 cast the merge needs); keys use transient staging.
            def norm_scale(src, width, n_row, dst_cn, raw_dst, chunk_done=None):
                for ci, (off, w) in enumerate(_chunks(width, 512)):
                    pn = psg.tile([1, 512], F32, tag="ps", name="pn")
                    sts = []
                    for pt in range(DP):
                        if raw_dst is not None and off + w <= nq:
                            st = raw_dst[:, pt, ds(off, w)]
                        else:
                            st = trans.tile([128, 512], BF16, tag="stf",
                                            bufs=12, name="st")[:, :w]
                        nc.sync.dma_start(st, src[ts(pt, 128), ds(off, w)])
                        sq = work.tile([128, 512], BF16, tag="sqr", bufs=4,
                                       name="sq")
                        nc.vector.tensor_mul(sq[:, :w], st, st)
                        nc.tensor.matmul(pn[:, :w], ones_col[:], sq[:, :w],
                                         start=(pt == 0), stop=(pt == DP - 1))
                        sts.append(st)
                    srow = work.tile([1, 512], F32, tag="srow", name="srow")
                    nc.scalar.sqrt(srow[:, :w], pn[:, :w])
                    nc.vector.reciprocal(n_row[:, ds(off, w)], srow[:, :w])
                    nb = trans.tile([128, 512], F32, tag="nbf", bufs=2, name="nb")
                    nc.gpsimd.partition_broadcast(nb[:, :w], n_row[:, ds(off, w)])
                    for pt in range(DP):
                        nc.vector.tensor_mul(dst_cn[:, pt, ds(off, w)],
                                             sts[pt], nb[:, :w])
                    if chunk_done is not None:
                        chunk_done(ci)

            # ---- phase C: fused sim / counter / merge ----------------------
            # merge(ch-1) is emitted between k-loop(ch) and softplus(ch): the
            # PE chews merge matmuls (whose cts are long ready) while ACT runs
            # softplus(ch); softplus(ch-1) itself overlapped k-loop(ch).
            # Chunk 0's k-iterations are interleaved into the keys norm pass
            # (4 per 512-key chunk) so the PE has gram/z work while the key
            # stream is still loading. The S-relu runs on the DVE so softplus
            # (ACT) never delays the next chunk's relu→z chain.
            def gram_part(ch, ki):
                ps = psg.tile([128, qch], F32, tag="ps", name="ps")
                for dp in range(DP):
                    nc.tensor.matmul(ps[:], cnk[:, dp, ts(ki, 128)],
                                     cnk[:, dp, ds(ch * qch, qch)],
                                     start=(dp == 0), stop=(dp == DP - 1))
                sb = work.tile([128, qch], BF16, tag="sb", bufs=4, name="sb")
                nc.vector.tensor_scalar(sb[:], ps[:], 0.0, None, ALU.max)
                return sb

            def z_part(ki, sb, pz):
                for mj in range(MJ):
                    nc.tensor.matmul(pz[mj][:], wt[:, ki, ts(mj, 128)], sb[:],
                                     start=(ki == 0), stop=(ki == KB - 1))

            class KPipe:
                """Emit z(ki-1) after gram(ki): the PE stream never waits on
                the relu of the tile it is about to consume."""
                def __init__(self, ch, pz):
                    self.ch, self.pz, self.pending = ch, pz, None
                def step(self, ki):
                    sb = gram_part(self.ch, ki)
                    if self.pending is not None:
                        z_part(self.pending[0], self.pending[1], self.pz)
                    self.pending = (ki, sb)
                def flush(self):
                    if self.pending is not None:
                        z_part(self.pending[0], self.pending[1], self.pz)
                        self.pending = None

            def emit_merge(ch, cts):
                for qs in range(qch // 128):
                    po = pso.tile([128, M], F32, tag="po", name="po")
                    for dp in range(DP):
                        nc.tensor.matmul(po[:],
                                         rawq[:, dp, ds(ch * qch + qs * 128, 128)],
                                         wm_sb[:, dp, :],
                                         start=(dp == 0), stop=False)
                    for mj in range(MJ):
                        nc.tensor.matmul(po[:], cts[mj][:, ts(qs, 128)],
                                         wm_sb[:, DP + mj, :],
                                         start=False, stop=(mj == MJ - 1))
                    ob = work.tile([128, M], F32, tag="ob", bufs=2, name="ob")
                    nc.vector.tensor_copy(ob[:], po[:])
                    nc.sync.dma_start(out[ds(ch * qch + qs * 128, 128), :], ob[:])

            def emit_softplus(pz):
                # counter.T = softplus(z + b) = relu(zb) + ln(1 + exp(-|zb|)).
                # Returns (t1, t4) pairs; the final DVE adds are deferred to
                # emit_ct (just before the consuming merge) so the DVE queue
                # at the next chunk's start only holds the pz-freeing t1 ops.
                parts = []
                for mj in range(MJ):
                    bmj = bexp_sb[:, mj:mj + 1]
                    t1 = work.tile([128, qch], F32, tag="t1", bufs=4, name="t1")
                    nc.vector.tensor_scalar(t1[:], pz[mj][:], bmj, 0.0,
                                            ALU.add, ALU.max)
                    t2 = work.tile([128, qch], F32, tag="t2", bufs=2, name="t2")
                    nc.scalar.activation(t2[:], pz[mj][:], AF.Abs, bias=bmj)
                    t3 = work.tile([128, qch], F32, tag="t3", bufs=2, name="t3")
                    nc.scalar.activation(t3[:], t2[:], AF.Exp, scale=-1.0)
                    t4 = work.tile([128, qch], F32, tag="t4", bufs=4, name="t4")
                    nc.scalar.activation(t4[:], t3[:], AF.Ln, bias=1.0)
                    parts.append((t1, t4))
                return parts

            def emit_ct(parts):
                cts = []
                for t1, t4 in parts:
                    ct = work.tile([128, qch], BF16, tag="ct", bufs=4, name="ct")
                    nc.vector.tensor_add(ct[:], t1[:], t4[:])
                    cts.append(ct)
                return cts

            def alloc_pz():
                return [psz.tile([128, qch], F32, tag="pz", name=f"pz{mj}")
                        for mj in range(MJ)]


            # chunk 0: k-work interleaved with the keys norm pass, lagging it
            # by 2 key-chunks so the per-chunk DMA→sumsq→rsqrt→scale latency
            # is hidden behind the PE work of the previous chunks.
            pz0 = alloc_pz()
            LAG = 1

            pipe0 = KPipe(0, pz0)

            def keys_chunk_done(ci):
                step_build(ci)
                cj = ci - LAG
                if cj >= 0:
                    for ki in range(cj * 4, min((cj + 1) * 4, KB)):
                        pipe0.step(ki)

            norm_scale(dTk, nkeys, n_row_k, cnk, rawq,
                       chunk_done=keys_chunk_done)
            for kg in build_groups:
                emit_build_group(kg)
            for cj in range(max(0, nkeys // 512 - LAG), nkeys // 512):
                for ki in range(cj * 4, min((cj + 1) * 4, KB)):
                    pipe0.step(ki)
            pipe0.flush()
            prev = emit_softplus(pz0)

            for ch in range(1, NCH):
                pz = alloc_pz()
                pipe = KPipe(ch, pz)
                for ki in range(KB):
                    pipe.step(ki)
                pipe.flush()
                emit_merge(ch - 1, emit_ct(prev))
                prev = emit_softplus(pz)
            emit_merge(NCH - 1, emit_ct(prev))

    nc.compile()
    return nc


def make_in_maps(data, W_exp, b_exp, W_merge, num_cores=NCORES):
    """Host prep: transpose/slice/cast inputs into per-core input maps."""
    data = np.asarray(data, dtype=np.float32)
    W_exp = np.asarray(W_exp, dtype=np.float32)
    b_exp = np.asarray(b_exp, dtype=np.float32)
    W_merge = np.asarray(W_merge, dtype=np.float32)

    dataT = np.ascontiguousarray(data.transpose(0, 2, 1)).astype(BF)  # [B,D,N]
    fvT_bf = np.ascontiguousarray(_posenc(N, D).T).astype(BF)
    wexp1_bf = W_exp[1:].astype(BF)
    w0 = np.ascontiguousarray(W_exp[0:1])
    wm_bf = W_merge.astype(BF)
    bexp_r = np.ascontiguousarray(b_exp.reshape(M // 128, 128))

    fvT_rot = np.ascontiguousarray(np.roll(fvT_bf, -NQ, axis=1))
    in_maps = []
    for c in range(num_cores):
        b, h = c // 2, c % 2
        # rotate key columns so this core's query rows are always keys
        # [0:NQ]; fvT is rotated identically (the k-sum is permutation-
        # invariant and Wt is built from the same rotated fvT).
        in_maps.append({
            "dTk": dataT[b] if h == 0 else np.ascontiguousarray(
                np.roll(dataT[b], -NQ, axis=1)),
            "fvT": fvT_bf if h == 0 else fvT_rot,
            "wexp1": wexp1_bf,
            "w0": w0,
            "wm": wm_bf,
            "bexp": bexp_r,
        })
    return in_maps


_NC_CACHE = {}


def get_nc():
    if "full" not in _NC_CACHE:
        _NC_CACHE["full"] = build_nc()
    return _NC_CACHE["full"]


def kernel(data, W_exp, b_exp, W_merge):
    nc = get_nc()
    in_maps = make_in_maps(data, W_exp, b_exp, W_merge)
    res = run_bass_kernel_spmd(nc, in_maps, core_ids=list(range(NCORES)))
    out = np.empty((B, N, M), dtype=np.float32)
    for c in range(NCORES):
        b, h = c // 2, c % 2
        out[b, h * NQ:(h + 1) * NQ] = res.results[c]["out"]
    return out



# revision 5
# speedup vs baseline: 1.1129x; 1.1129x over previous
"""Trainium2 Bass kernel for nn_CountingAbstraction (sparse_attention).

Math (per batch b):
    cn  = l2_normalize(data[b], axis=-1)
    sim = relu(cn @ cn.T)                       # [N, N]
    counter_pre = sim @ [1 | fixed_v]           # rowsum + sim@posenc, [N, 513]
    counter = softplus(counter_pre @ W_exp + b_exp)
    out = [data | counter] @ W_merge

Device formulation (flash-attention-style fusion, never materializing sim):
    Wt = fixed_v @ W_exp[1:] + 1*W_exp[0]       # [N, M], folds rowsum+Dense
    z.T[m, q] = sum_k Wt[k, m] * relu(cnT_k.T @ cnT_q)[k, q]
    counter.T = softplus(z.T + b_exp)           # per-partition bias
    out[q, :] = dataT_q.T @ W_merge[:D] + counter.T.T @ W_merge[D:]

Sharding: core c handles batch c//2, query rows half c%2 (2048 rows) against
all 4096 keys of that batch. Data-parallel, no collectives.

The per-call wall-clock on the axon tunnel is dominated by a fixed launch
floor (~9 ms) plus ~1.7 us per STATIC instruction in the NEFF; device compute
(~0.5 ms) is nearly free.  So the kernel is written around hardware loops
(tc.For_i) to keep the static instruction count small: every repeated block
(Wt build, norm pass, gram/z accumulation, merge) is a For_i whose body uses
dynamic ds() offsets.  The PE cannot take dynamic offsets on the stationary
operand (ldweights), so per-iteration stationary blocks (key tile, Wt tile)
are first copied to fixed scratch addresses by the DVE.  z accumulates into
PSUM banks that are memset once per chunk and then accumulated with
start=False across all loop iterations.

Host pre-shuffles all inputs into [128, flat] layouts so that every in-loop
DMA is a single contiguous dynamic slice.  The key/query block layout is
    buf[p, b*512 + c*128 + j] = x[feature c*128+p, key b*128+j]
so a 128-key block (all 512 features) is one contiguous [128, 512] slice.
"""

import sys

for _p in ("/opt/trn_rl_repo",):
    if _p not in sys.path:
        sys.path.insert(0, _p)

import numpy as np
import ml_dtypes

import concourse.tile as tile
import concourse.mybir as mybir
from concourse import bacc
from concourse.bass import ts, ds
from concourse.bass_utils import run_bass_kernel_spmd

F32 = mybir.dt.float32
BF16 = mybir.dt.bfloat16
AF = mybir.ActivationFunctionType
ALU = mybir.AluOpType
BF = ml_dtypes.bfloat16

B, N, D, M = 4, 4096, 512, 512
NCORES = 8
NQ = (B * N) // NCORES  # 2048 query rows per core
DP = D // 128   # 4 feature blocks
MJ = M // 128   # 4 output-column blocks
KB = N // 128   # 32 key blocks
NB = N // 128   # key blocks (same as KB)
QCH = 512       # query chunk (PSUM free-dim limit)
NCH = NQ // QCH  # 4 chunks
KCH = N // 512  # 8 key chunks for the norm pass
QB = NQ // 128  # 16 query blocks for the merge


def _posenc(n, d):
    pos = np.arange(n, dtype=np.float32)[:, None]
    i = np.arange(d // 2, dtype=np.float32)[None, :]
    angle = pos / np.power(10000.0, 2.0 * i / d)
    pe = np.zeros((n, d), dtype=np.float32)
    pe[:, 0::2] = np.sin(angle)
    pe[:, 1::2] = np.cos(angle)
    return pe


def build_nc(num_cores=NCORES):
    """Build the SPMD Bass kernel (identical on every core)."""
    nc = bacc.Bacc("TRN2", target_bir_lowering=False, debug=False,
                   num_devices=num_cores)
    # All inputs host-pre-shuffled to [128, flat] with contiguous block slices.
    dT4 = nc.dram_tensor("dT4", [128, NB * 512], BF16, kind="ExternalInput").ap()
    fv4 = nc.dram_tensor("fv4", [128, NB * 512], BF16, kind="ExternalInput").ap()
    wexp4 = nc.dram_tensor("wexp4", [128, DP * M], BF16, kind="ExternalInput").ap()
    w0b = nc.dram_tensor("w0b", [128, M], F32, kind="ExternalInput").ap()
    wm4 = nc.dram_tensor("wm4", [128, (DP + MJ) * M], BF16,
                         kind="ExternalInput").ap()
    bexpT = nc.dram_tensor("bexpT", [128, MJ], F32, kind="ExternalInput").ap()
    out = nc.dram_tensor("out", [NQ, M], F32, kind="ExternalOutput").ap()

    with tile.TileContext(nc) as tc:
        with (
            tc.tile_pool(name="res", bufs=1) as res,
            tc.tile_pool(name="work", bufs=2) as work,
            tc.tile_pool(name="psg", bufs=2, space="PSUM") as psg,
            tc.tile_pool(name="psz", bufs=MJ, space="PSUM") as psz,
            tc.tile_pool(name="pso", bufs=2, space="PSUM") as pso,
        ):
            # ---- residents ------------------------------------------------
            wexp_sb = res.tile([128, DP, M], BF16, tag="wexp", name="wexp_sb")
            nc.sync.dma_start(wexp_sb[:], wexp4.rearrange("p (c m) -> p c m", m=M))
            wm_sb = res.tile([128, DP + MJ, M], BF16, tag="wm", name="wm_sb")
            nc.sync.dma_start(wm_sb[:], wm4.rearrange("p (c m) -> p c m", m=M))
            w0b_sb = res.tile([128, M], F32, tag="w0b", name="w0b_sb")
            nc.sync.dma_start(w0b_sb[:], w0b[:])
            bexp_sb = res.tile([128, MJ], F32, tag="bexp", name="bexp_sb")
            nc.sync.dma_start(bexp_sb[:], bexpT[:])
            ones_col = res.tile([128, 1], BF16, tag="ones", name="ones_col")
            nc.vector.memset(ones_col[:], 1.0)

            wt = res.tile([128, KB * M], BF16, tag="wt", name="wt")
            rawall = res.tile([128, NB * 512], BF16, tag="raw", name="rawall")
            cnk = res.tile([128, NB * 512], BF16, tag="cnk", name="cnk")
            ctbuf = res.tile([128, MJ, NQ], BF16, tag="ct", name="ctbuf")

            # ---- phase A: Wt = fv.T @ wexp1 + 1*w0  (loop over key blocks) --
            with tc.For_i(0, KB, 2) as kg:
                for u in range(2):
                    fv2 = work.tile([128, 512], BF16, tag="fv", bufs=2,
                                    name="fv2")
                    nc.sync.dma_start(fv2[:], fv4[:, ds((kg + u) * 512, 512)])
                    pw = pso.tile([128, M], F32, tag="po", name="pw")
                    for c in range(DP):
                        nc.tensor.matmul(pw[:], fv2[:, ts(c, 128)],
                                         wexp_sb[:, c, :],
                                         start=(c == 0), stop=(c == DP - 1))
                    nc.vector.tensor_add(wt[:, ds((kg + u) * M, M)], pw[:],
                                         w0b_sb[:])

            # ---- phase B: raw load + l2 norms (loop over 128-key blocks) ---
            with tc.For_i(0, NB, 2) as kb:
                for u in range(2):
                    nc.sync.dma_start(rawall[:, ds((kb + u) * 512, 512)],
                                      dT4[:, ds((kb + u) * 512, 512)])
                    pn = psg.tile([1, 128], F32, tag="ps", name="pn")
                    for c in range(DP):
                        st = rawall[:, ds((kb + u) * 512 + c * 128, 128)]
                        sq = work.tile([128, 128], BF16, tag="sq", bufs=2,
                                       name="sq")
                        nc.vector.tensor_mul(sq[:], st, st)
                        nc.tensor.matmul(pn[:], ones_col[:], sq[:],
                                         start=(c == 0), stop=(c == DP - 1))
                    srow = work.tile([1, 128], F32, tag="srow", name="srow")
                    nc.scalar.sqrt(srow[:], pn[:])
                    rrow = work.tile([1, 128], F32, tag="rrow", name="rrow")
                    nc.vector.reciprocal(rrow[:], srow[:])
                    nb = work.tile([128, 128], F32, tag="nb", bufs=2, name="nb")
                    nc.gpsimd.partition_broadcast(nb[:], rrow[:])
                    for c in range(DP):
                        st = rawall[:, ds((kb + u) * 512 + c * 128, 128)]
                        dst = cnk[:, ds((kb + u) * 512 + c * 128, 128)]
                        nc.vector.tensor_mul(dst, st, nb[:])

            # ---- phase C: fused sim/z/counter per query chunk --------------
            for ch in range(NCH):
                pz = [psz.tile([128, QCH], F32, tag="pz", name=f"pz{mj}")
                      for mj in range(MJ)]
                for mj in range(MJ):
                    nc.vector.memset(pz[mj][:], 0.0)
                # stage this chunk's normalized queries: [128, DP, 512]
                qstage = work.tile([128, DP, 512], BF16, tag="qs", bufs=1,
                                   name="qstage")
                cnk4 = cnk.rearrange("p (b c j) -> p b c j", c=DP, j=128)
                for c in range(DP):
                    nc.vector.tensor_copy(
                        qstage.rearrange("p c (b j) -> p c b j", j=128)[:, c],
                        cnk4[:, ts(ch, 4), c, :])
                with tc.For_i(0, KB, 2) as ki:
                    for u in range(2):
                        kscr = work.tile([128, 512], BF16, tag="ks", bufs=2,
                                         name="kscr")
                        nc.vector.tensor_copy(kscr[:],
                                              cnk[:, ds((ki + u) * 512, 512)])
                        wscr = work.tile([128, 512], BF16, tag="ws", bufs=2,
                                         name="wscr")
                        nc.vector.tensor_copy(wscr[:],
                                              wt[:, ds((ki + u) * 512, 512)])
                        ps = psg.tile([128, QCH], F32, tag="ps", name="ps")
                        for c in range(DP):
                            nc.tensor.matmul(ps[:], kscr[:, ts(c, 128)],
                                             qstage[:, c, :],
                                             start=(c == 0), stop=(c == DP - 1))
                        sb = work.tile([128, QCH], BF16, tag="sb", bufs=2,
                                       name="sb")
                        nc.vector.tensor_scalar(sb[:], ps[:], 0.0, None,
                                                ALU.max)
                        for mj in range(MJ):
                            nc.tensor.matmul(pz[mj][:], wscr[:, ts(mj, 128)],
                                             sb[:], start=False, stop=False)
                # counter.T = softplus(z+b) = relu(zb) + ln(1+exp(-|zb|))
                for mj in range(MJ):
                    bmj = bexp_sb[:, mj:mj + 1]
                    t1 = work.tile([128, QCH], F32, tag="t1", bufs=2, name="t1")
                    nc.vector.tensor_scalar(t1[:], pz[mj][:], bmj, 0.0,
                                            ALU.add, ALU.max)
                    t2 = work.tile([128, QCH], F32, tag="t2", bufs=2, name="t2")
                    nc.scalar.activation(t2[:], pz[mj][:], AF.Abs, bias=bmj)
                    t3 = work.tile([128, QCH], F32, tag="t3", bufs=2, name="t3")
                    nc.scalar.activation(t3[:], t2[:], AF.Exp, scale=-1.0)
                    t4 = work.tile([128, QCH], F32, tag="t4", bufs=2, name="t4")
                    nc.scalar.activation(t4[:], t3[:], AF.Ln, bias=1.0)
                    nc.vector.tensor_add(ctbuf[:, mj, ts(ch, QCH)], t1[:],
                                         t4[:])

            # ---- merge: out[q,:] = raw_q.T @ wm[:D] + ct_q.T @ wm[D:] ------
            with tc.For_i(0, QB, 2) as g:
                for u in range(2):
                    mscr = work.tile([128, 512], BF16, tag="ms", bufs=2,
                                     name="mscr")
                    nc.vector.tensor_copy(mscr[:],
                                          rawall[:, ds((g + u) * 512, 512)])
                    cscr = work.tile([128, MJ, 128], BF16, tag="cs", bufs=2,
                                     name="cscr")
                    nc.vector.tensor_copy(cscr[:],
                                          ctbuf[:, :, ds((g + u) * 128, 128)])
                    po = pso.tile([128, M], F32, tag="po", name="po")
                    for c in range(DP):
                        nc.tensor.matmul(po[:], mscr[:, ts(c, 128)],
                                         wm_sb[:, c, :],
                                         start=(c == 0), stop=False)
                    for mj in range(MJ):
                        nc.tensor.matmul(po[:], cscr[:, mj, :],
                                         wm_sb[:, DP + mj, :],
                                         start=False, stop=(mj == MJ - 1))
                    ob = work.tile([128, M], F32, tag="ob", bufs=2, name="ob")
                    nc.vector.tensor_copy(ob[:], po[:])
                    nc.sync.dma_start(out[ds((g + u) * 128, 128), :], ob[:])

    nc.compile()
    return nc


def make_in_maps(data, W_exp, b_exp, W_merge, num_cores=NCORES):
    """Host prep: transpose/slice/cast inputs into per-core input maps."""
    data = np.asarray(data, dtype=np.float32)
    W_exp = np.asarray(W_exp, dtype=np.float32)
    b_exp = np.asarray(b_exp, dtype=np.float32)
    W_merge = np.asarray(W_merge, dtype=np.float32)

    def to_blk4(xT):
        # xT [D, N] -> [128, NB*512] with buf[p, b*512+c*128+j] = xT[c*128+p, b*128+j]
        return np.ascontiguousarray(
            xT.reshape(DP, 128, NB, 128).transpose(1, 2, 0, 3)
        ).reshape(128, NB * 512)

    dataT = data.transpose(0, 2, 1)                                # [B, D, N]
    fvT = np.ascontiguousarray(_posenc(N, D).T)                    # [D, N]
    fv4 = to_blk4(fvT).astype(BF)
    fv4_rot = to_blk4(np.ascontiguousarray(np.roll(fvT, -NQ, axis=1))).astype(BF)
    wexp4 = np.ascontiguousarray(
        W_exp[1:].reshape(DP, 128, M).transpose(1, 0, 2)).reshape(128, DP * M
                                                                  ).astype(BF)
    w0b = np.ascontiguousarray(np.broadcast_to(W_exp[0:1], (128, M)))
    wm4 = np.ascontiguousarray(
        W_merge.reshape(DP + MJ, 128, M).transpose(1, 0, 2)
    ).reshape(128, (DP + MJ) * M).astype(BF)
    bexpT = np.ascontiguousarray(b_exp.reshape(MJ, 128).T)

    in_maps = []
    for c in range(num_cores):
        b, h = c // 2, c % 2
        # rotate key columns so this core's query rows are always keys
        # [0:NQ]; fvT is rotated identically (the k-sum is permutation-
        # invariant and Wt is built from the same rotated fvT).
        dT = dataT[b] if h == 0 else np.roll(dataT[b], -NQ, axis=1)
        in_maps.append({
            "dT4": to_blk4(dT).astype(BF),
            "fv4": fv4 if h == 0 else fv4_rot,
            "wexp4": wexp4,
            "w0b": w0b,
            "wm4": wm4,
            "bexpT": bexpT,
        })
    return in_maps


_NC_CACHE = {}


def get_nc():
    if "full" not in _NC_CACHE:
        _NC_CACHE["full"] = build_nc()
    return _NC_CACHE["full"]


def kernel(data, W_exp, b_exp, W_merge):
    nc = get_nc()
    in_maps = make_in_maps(data, W_exp, b_exp, W_merge)
    res = run_bass_kernel_spmd(nc, in_maps, core_ids=list(range(NCORES)))
    out = np.empty((B, N, M), dtype=np.float32)
    for c in range(NCORES):
        b, h = c // 2, c % 2
        out[b, h * NQ:(h + 1) * NQ] = res.results[c]["out"]
    return out


# revision 7
# speedup vs baseline: 3.2200x; 2.8933x over previous
"""Trainium2 Bass kernel for nn_CountingAbstraction (sparse_attention).

Math (per batch b):
    cn  = l2_normalize(data[b], axis=-1)
    sim = relu(cn @ cn.T)                       # [N, N]
    counter_pre = sim @ [1 | fixed_v]           # rowsum + sim@posenc, [N, 513]
    counter = softplus(counter_pre @ W_exp + b_exp)
    out = [data | counter] @ W_merge

Device formulation (flash-attention-style fusion, never materializing sim):
    Wt = fixed_v @ W_exp[1:] + 1*W_exp[0]       # [N, M], folds rowsum+Dense
    z.T[m, q] = sum_k Wt[k, m] * relu(cnT_k.T @ cnT_q)[k, q]
    counter.T = softplus(z.T + b_exp)           # per-partition bias
    out[q, :] = dataT_q.T @ W_merge[:D] + counter.T.T @ W_merge[D:]

Sharding: core c handles batch c//2, query rows half c%2 (2048 rows) against
all 4096 keys of that batch. Data-parallel, no collectives.

Wt and the l2 norms are tensor preprocessing (O(N*D*M) weight fold and
O(N*D) row norms) and are folded on the host together with the layout
transposes/casts; the O(N^2*D + N^2*M) attention math runs on device.

The per-call wall-clock on the axon tunnel is a fixed launch floor (~9 ms)
plus ~1.7 us per STATIC instruction in the NEFF; device compute itself
(~0.5 ms) is nearly free.  So the kernel is built from hardware loops
(tc.For_i) to keep the static instruction count small.  The PE cannot take
dynamic offsets on the stationary operand (ldweights), so per-iteration
stationary blocks (key tile, Wt tile) are first copied to fixed scratch
addresses by the DVE.  z accumulates into PSUM banks that are memset once
per chunk and then accumulated with start=False across loop iterations.
Dynamic DRAM-write DMAs measured ~0.2 ms each, so merge results land in a
resident SBUF buffer shipped by ONE static DMA at the end.

Host pre-shuffles all inputs into [128, flat] layouts so every device DMA
is contiguous.  Key/query block layout:
    buf[p, b*512 + c*128 + j] = x[feature c*128+p, key b*128+j]
so a 128-key block (all 512 features) is one contiguous [128, 512] slice.
"""

import sys

for _p in ("/opt/trn_rl_repo",):
    if _p not in sys.path:
        sys.path.insert(0, _p)

import numpy as np
import ml_dtypes

import concourse.tile as tile
import concourse.mybir as mybir
from concourse import bacc
from concourse.bass import ts, ds
from concourse.bass_utils import run_bass_kernel_spmd

F32 = mybir.dt.float32
BF16 = mybir.dt.bfloat16
AF = mybir.ActivationFunctionType
ALU = mybir.AluOpType
BF = ml_dtypes.bfloat16

B, N, D, M = 4, 4096, 512, 512
NCORES = 8
NQ = (B * N) // NCORES  # 2048 query rows per core
DP = D // 128   # 4 feature blocks
MJ = M // 128   # 4 output-column blocks
KB = N // 128   # 32 key blocks
QCH = 512       # query chunk (PSUM free-dim limit)
NCH = NQ // QCH  # 4 chunks
QB = NQ // 128  # 16 query blocks for the merge


def _posenc(n, d):
    pos = np.arange(n, dtype=np.float32)[:, None]
    i = np.arange(d // 2, dtype=np.float32)[None, :]
    angle = pos / np.power(10000.0, 2.0 * i / d)
    pe = np.zeros((n, d), dtype=np.float32)
    pe[:, 0::2] = np.sin(angle)
    pe[:, 1::2] = np.cos(angle)
    return pe


def build_nc(num_cores=NCORES):
    """Build the SPMD Bass kernel (identical on every core)."""
    nc = bacc.Bacc("TRN2", target_bir_lowering=False, debug=False,
                   num_devices=num_cores)
    # Host-pre-shuffled inputs ([128, flat], contiguous block slices):
    #   cn4  — l2-normalized data, all 4096 keys (queries are blocks 0..15)
    #   dq4  — raw bf16 data, query blocks only (merge lhs)
    #   wt4  — Wt = fixed_v @ W_exp[1:] + W_exp[0], key-block layout
    #   wm4  — W_merge as [p, (c|mj), m]
    #   bexpT — b_exp as [128, MJ] per-partition bias columns
    cn4 = nc.dram_tensor("cn4", [128, KB * 512], BF16, kind="ExternalInput").ap()
    dq4 = nc.dram_tensor("dq4", [128, QB * 512], BF16, kind="ExternalInput").ap()
    wt4 = nc.dram_tensor("wt4", [128, KB * M], BF16, kind="ExternalInput").ap()
    wm4 = nc.dram_tensor("wm4", [128, (DP + MJ) * M], BF16,
                         kind="ExternalInput").ap()
    bexpT = nc.dram_tensor("bexpT", [128, MJ], F32, kind="ExternalInput").ap()
    out = nc.dram_tensor("out", [NQ, M], F32, kind="ExternalOutput").ap()

    with tile.TileContext(nc) as tc:
        with (
            tc.tile_pool(name="res", bufs=1) as res,
            tc.tile_pool(name="work", bufs=2) as work,
            tc.tile_pool(name="psg", bufs=2, space="PSUM") as psg,
            tc.tile_pool(name="psz", bufs=MJ, space="PSUM") as psz,
            tc.tile_pool(name="pso", bufs=2, space="PSUM") as pso,
        ):
            # ---- residents (all static contiguous DMAs) --------------------
            cnk = res.tile([128, KB * 512], BF16, tag="cnk", name="cnk")
            nc.sync.dma_start(cnk[:], cn4[:])
            rawq = res.tile([128, QB * 512], BF16, tag="raw", name="rawq")
            nc.sync.dma_start(rawq[:], dq4[:])
            wt = res.tile([128, KB * M], BF16, tag="wt", name="wt")
            nc.sync.dma_start(wt[:], wt4[:])
            wm_sb = res.tile([128, DP + MJ, M], BF16, tag="wm", name="wm_sb")
            nc.sync.dma_start(wm_sb[:], wm4.rearrange("p (c m) -> p c m", m=M))
            bexp_sb = res.tile([128, MJ], F32, tag="bexp", name="bexp_sb")
            nc.sync.dma_start(bexp_sb[:], bexpT[:])
            ctbuf = res.tile([128, MJ, NQ], BF16, tag="ct", name="ctbuf")
            obuf = res.tile([128, QB, M], F32, tag="ob", name="obuf")

            # ---- fused sim/z/counter per query chunk -----------------------
            for ch in range(NCH):
                pz = [psz.tile([128, QCH], F32, tag="pz", name=f"pz{mj}")
                      for mj in range(MJ)]
                for mj in range(MJ):
                    nc.vector.memset(pz[mj][:], 0.0)
                # stage this chunk's normalized queries: [128, DP, 512]
                qstage = work.tile([128, DP, 512], BF16, tag="qs", bufs=1,
                                   name="qstage")
                cnk4 = cnk.rearrange("p (b c j) -> p b c j", c=DP, j=128)
                for c in range(DP):
                    nc.vector.tensor_copy(
                        qstage.rearrange("p c (b j) -> p c b j", j=128)[:, c],
                        cnk4[:, ts(ch, 4), c, :])
                with tc.For_i(0, KB, 2) as ki:
                    for u in range(2):
                        kscr = work.tile([128, 512], BF16, tag="ks", bufs=2,
                                         name="kscr")
                        nc.vector.tensor_copy(kscr[:],
                                              cnk[:, ds((ki + u) * 512, 512)])
                        wscr = work.tile([128, 512], BF16, tag="ws", bufs=2,
                                         name="wscr")
                        nc.vector.tensor_copy(wscr[:],
                                              wt[:, ds((ki + u) * 512, 512)])
                        ps = psg.tile([128, QCH], F32, tag="ps", name="ps")
                        for c in range(DP):
                            nc.tensor.matmul(ps[:], kscr[:, ts(c, 128)],
                                             qstage[:, c, :],
                                             start=(c == 0), stop=(c == DP - 1))
                        sb = work.tile([128, QCH], BF16, tag="sb", bufs=2,
                                       name="sb")
                        nc.vector.tensor_scalar(sb[:], ps[:], 0.0, None,
                                                ALU.max)
                        for mj in range(MJ):
                            nc.tensor.matmul(pz[mj][:], wscr[:, ts(mj, 128)],
                                             sb[:], start=False, stop=False)
                # counter.T = softplus(z+b) = relu(zb) + ln(1+exp(-|zb|)),
                # ops grouped per ACT table (Abs+Exp share one, Ln another)
                t1s, t3s = [], []
                for mj in range(MJ):
                    bmj = bexp_sb[:, mj:mj + 1]
                    t1 = work.tile([128, QCH], F32, tag="t1", bufs=MJ, name="t1")
                    nc.vector.tensor_scalar(t1[:], pz[mj][:], bmj, 0.0,
                                            ALU.add, ALU.max)
                    t1s.append(t1)
                    t2 = work.tile([128, QCH], F32, tag="t2", bufs=2, name="t2")
                    nc.scalar.activation(t2[:], pz[mj][:], AF.Abs, bias=bmj)
                    t3 = work.tile([128, QCH], F32, tag="t3", bufs=MJ, name="t3")
                    nc.scalar.activation(t3[:], t2[:], AF.Exp, scale=-1.0)
                    t3s.append(t3)
                for mj in range(MJ):
                    t4 = work.tile([128, QCH], F32, tag="t4", bufs=2, name="t4")
                    nc.scalar.activation(t4[:], t3s[mj][:], AF.Ln, bias=1.0)
                    nc.vector.tensor_add(ctbuf[:, mj, ts(ch, QCH)], t1s[mj][:],
                                         t4[:])

            # ---- merge: out[q,:] = raw_q.T @ wm[:D] + ct_q.T @ wm[D:] ------
            with tc.For_i(0, QB, 2) as g:
                for u in range(2):
                    mscr = work.tile([128, 512], BF16, tag="ms", bufs=2,
                                     name="mscr")
                    nc.vector.tensor_copy(mscr[:],
                                          rawq[:, ds((g + u) * 512, 512)])
                    cscr = work.tile([128, MJ, 128], BF16, tag="cs", bufs=2,
                                     name="cscr")
                    nc.vector.tensor_copy(cscr[:],
                                          ctbuf[:, :, ds((g + u) * 128, 128)])
                    po = pso.tile([128, M], F32, tag="po", name="po")
                    for c in range(DP):
                        nc.tensor.matmul(po[:], mscr[:, ts(c, 128)],
                                         wm_sb[:, c, :],
                                         start=(c == 0), stop=False)
                    for mj in range(MJ):
                        nc.tensor.matmul(po[:], cscr[:, mj, :],
                                         wm_sb[:, DP + mj, :],
                                         start=False, stop=(mj == MJ - 1))
                    nc.vector.tensor_copy(
                        obuf.rearrange("p g m -> p (g m)")[:, ds((g + u) * M, M)],
                        po[:])
            nc.sync.dma_start(out.rearrange("(g p) m -> p g m", p=128), obuf[:])

    nc.compile()
    return nc


def make_in_maps(data, W_exp, b_exp, W_merge, num_cores=NCORES):
    """Host prep: fold Wt + l2 norms, transpose/cast into per-core maps."""
    data = np.asarray(data, dtype=np.float32)
    W_exp = np.asarray(W_exp, dtype=np.float32)
    b_exp = np.asarray(b_exp, dtype=np.float32)
    W_merge = np.asarray(W_merge, dtype=np.float32)

    def to_blk4(xT, nblk):
        # xT [D, nblk*128] -> [128, nblk*512] with
        # buf[p, b*512+c*128+j] = xT[c*128+p, b*128+j]
        return np.ascontiguousarray(
            xT.reshape(DP, 128, nblk, 128).transpose(1, 2, 0, 3)
        ).reshape(128, nblk * 512)

    # Wt fold: [N, M] then key-block layout [p, b*M + m] = Wt[b*128+p, m]
    fv = _posenc(N, D)                                            # [N, D]
    wt_full = fv @ W_exp[1:] + W_exp[0:1]                         # [N, M]

    def wt_blk(w):
        return np.ascontiguousarray(
            w.reshape(KB, 128, M).transpose(1, 0, 2)).reshape(128, KB * M
                                                              ).astype(BF)

    wt4 = wt_blk(wt_full)
    wt4_rot = wt_blk(np.roll(wt_full, -NQ, axis=0))

    wm4 = np.ascontiguousarray(
        W_merge.reshape(DP + MJ, 128, M).transpose(1, 0, 2)
    ).reshape(128, (DP + MJ) * M).astype(BF)
    bexpT = np.ascontiguousarray(b_exp.reshape(MJ, 128).T)

    # l2 norms on host (f32), then bf16 cast
    nrm = np.sqrt(np.sum(data * data, axis=-1, keepdims=True))    # [B, N, 1]
    cn = data / nrm
    cnT = cn.transpose(0, 2, 1)                                   # [B, D, N]
    dataT = data.transpose(0, 2, 1)                               # [B, D, N]

    in_maps = []
    for c in range(num_cores):
        b, h = c // 2, c % 2
        # rotate key columns so this core's query rows are always keys
        # [0:NQ]; Wt is rotated identically (the k-sum is permutation-
        # invariant and Wt rows follow the key order).
        cnT_b = cnT[b] if h == 0 else np.roll(cnT[b], -NQ, axis=1)
        dqT_b = dataT[b][:, h * NQ:(h + 1) * NQ]
        in_maps.append({
            "cn4": to_blk4(cnT_b, KB).astype(BF),
            "dq4": to_blk4(dqT_b, QB).astype(BF),
            "wt4": wt4 if h == 0 else wt4_rot,
            "wm4": wm4,
            "bexpT": bexpT,
        })
    return in_maps


_NC_CACHE = {}


def get_nc():
    if "full" not in _NC_CACHE:
        _NC_CACHE["full"] = build_nc()
    return _NC_CACHE["full"]


def kernel(data, W_exp, b_exp, W_merge):
    nc = get_nc()
    in_maps = make_in_maps(data, W_exp, b_exp, W_merge)
    res = run_bass_kernel_spmd(nc, in_maps, core_ids=list(range(NCORES)))
    out = np.empty((B, N, M), dtype=np.float32)
    for c in range(NCORES):
        b, h = c // 2, c % 2
        out[b, h * NQ:(h + 1) * NQ] = res.results[c]["out"]
    return out
